# revision 20
# baseline (speedup 1.0000x reference)
"""Block-causal attention Trainium2 kernel (8 NeuronCores).

Sharding: core c = b*4 + g handles batch b (of 2) and head-group g (4 of 16
heads). Each core computes the qkv projection, rmsnorm + 2-D RoPE,
block-causal attention and a partial output projection for its 256 channels;
the host sums the 4 per-group partials per batch.

On-chip layouts (per core):
  Q^T/K^T: feature-on-partition tiles QR/QI/KR/KI [128, 2048]; row 32*hh+j
    <-> head hh, complex pair j (R = even orig dim 2j, I = odd 2j+1).
  V: [l, d] tiles per head [128, 16, 65] with an all-ones column 64 so the
    softmax denominator falls out of the M=65 PV matmul.
  Scores: S^T [keys=128, q=256] per (head, frame, ktile); block-causal means
    frame t only attends keys < 256*(t+1) -- no mask tensor anywhere.
  rmsnorm: r = rsqrt(mean(q^2)+eps) via weighted ones-matmul over partitions;
    q_scale/k_scale are folded into the projection weights; the k-side
    0.125*r_k is folded into exp()'s per-partition scale and the q-side r_q
    is multiplied into Q^T during RoPE. exp() needs no max-subtraction
    (|scores| <= 8 after rmsnorm).
All matmuls run in float32r (full PE rate, ~1e-4 component error).
"""

import os
import numpy as np

import concourse.bass as bass
import concourse.mybir as mybir
import concourse.tile as tile
from concourse import bacc
from concourse.bass_utils import run_bass_kernel_spmd

F32 = mybir.dt.float32
F32R = mybir.dt.float32r
BF16 = mybir.dt.bfloat16
AF = mybir.ActivationFunctionType
MUL = mybir.AluOpType.mult
ADD = mybir.AluOpType.add
SUB = mybir.AluOpType.subtract

B, T, NP, D, H = 2, 8, 256, 1024, 16
L = T * NP            # 2048
HD = 64               # head dim
HPG = 4               # heads per group (4 groups x 2 batches = 8 cores)
CPG = HPG * HD        # 256 channels per group
NDT = D // 128        # 8 d-tiles
NLC = L // 512        # 4 l-chunks
NLT = L // 128        # 16 l-tiles
EPS = 1e-6

_CACHE = {}


def _emit(nc, tc, ctx, xT, wqk, wv, wo, wvec, costab, sintab, out, skb):
    sing = ctx.enter_context(tc.tile_pool(name="sing", bufs=1))
    xp = ctx.enter_context(tc.tile_pool(name="xp", bufs=8))
    tmp = ctx.enter_context(tc.tile_pool(name="tmp", bufs=2))
    sqp = ctx.enter_context(tc.tile_pool(name="sqp", bufs=2))
    ptp = ctx.enter_context(tc.tile_pool(name="ptp", bufs=6))
    rbp = ctx.enter_context(tc.tile_pool(name="rbp", bufs=2))
    osb = ctx.enter_context(tc.tile_pool(name="osb", bufs=2))
    # PSUM: one shared transient pool (6 banks, single tag) + PV pool (2)
    pps = ctx.enter_context(tc.tile_pool(name="pps", bufs=8, space="PSUM"))
    ppv = pps

    # ---- persistent SBUF ----
    wqk_sb = sing.tile([128, NDT, 512], F32R)
    nc.sync.dma_start(out=wqk_sb[:], in_=wqk.rearrange("(t p) o -> p t o", p=128))
    wv_sb = sing.tile([128, NDT, CPG], BF16)
    nc.sync.dma_start(out=wv_sb[:], in_=wv.rearrange("(t p) o -> p t o", p=128))
    wo_sb = sing.tile([128, 2, D], F32R)
    nc.sync.dma_start(out=wo_sb[:], in_=wo.rearrange("(t p) o -> p t o", p=128))
    wvec_sb = sing.tile([128, 4], F32R)
    nc.sync.dma_start(out=wvec_sb[:], in_=wvec[:])
    cos_sb = sing.tile([128, L], F32)
    nc.sync.dma_start(out=cos_sb[:], in_=costab[:])
    sin_sb = sing.tile([128, L], F32)
    nc.sync.dma_start(out=sin_sb[:], in_=sintab[:])

    qk_sb = [sing.tile([128, L], BF16, name=f"qk{i}") for i in range(4)]
    rope_sb = [sing.tile([128, L], BF16, name=f"rope{i}") for i in range(4)]
    v_sb = [sing.tile([128, NLT, 65], BF16, name=f"v{h}") for h in range(HPG)]
    att_sb = [sing.tile([128, L], F32R, name=f"att{i}") for i in range(2)]
    ones_f32 = sing.tile([128, NLT, 1], F32)
    nc.vector.memset(ones_f32[:], 1.0)
    for h in range(HPG):
        nc.vector.tensor_copy(v_sb[h][:, :, 64:65], ones_f32[:])

    epsP = sing.tile([128, 1], F32)
    nc.vector.memset(epsP[:], EPS)
    eps64P = sing.tile([128, 1], F32)
    nc.vector.memset(eps64P[:], 64.0 * EPS)
    rcp = ctx.enter_context(tc.tile_pool(name="rcp", bufs=2))
    Rq = sing.tile([128, L], F32)
    Rn1 = sing.tile([128, L], F32)
    skT = sing.tile([128, NLT, 4], F32)
    rkT = sing.tile([128, NLT, 4], F32)

    QP = [sing.tile([128, L], BF16, name=f"qp{i}") for i in range(2)]
    KP = [sing.tile([128, L], BF16, name=f"kp{i}") for i in range(2)]
    QRr, QIr, KRr, KIr = rope_sb

    # ---- phase 1: projections + rms partition-sums ----
    for lc in range(NLC):
        ls = slice(lc * 512, (lc + 1) * 512)
        xt = []
        xtb = []
        for dt in range(NDT):
            x1 = xp.tile([128, 512], F32R, name=f"xt{dt}", tag="xt")
            nc.sync.dma_start(out=x1[:], in_=xT[dt * 128:(dt + 1) * 128, ls])
            xt.append(x1)
            x2 = xp.tile([128, 512], BF16, name=f"xtb{dt}", tag="xtb")
            nc.gpsimd.dma_start(out=x2[:], in_=xT[dt * 128:(dt + 1) * 128, ls])
            xtb.append(x2)
        for pair, rowbase, wcol in ((0, 0, 0), (2, 4, 2)):
            sqs = []
            for comp in range(2):           # R then I
                ot = pair + comp
                ps = pps.tile([128, 512], F32, name="qkps", tag="ps")
                for dt in range(NDT):
                    nc.tensor.matmul(ps[:], wqk_sb[:, dt, ot * 128:(ot + 1) * 128],
                                     xt[dt][:], start=(dt == 0), stop=(dt == NDT - 1))
                nc.vector.tensor_copy(qk_sb[ot][:, ls], ps[:])
                sq = sqp.tile([128, 512], F32R, tag="sq")
                nc.scalar.activation(sq[:], qk_sb[ot][:, ls], AF.Square)
                sqs.append(sq)
            for hh in range(HPG):
                r0 = 32 * hh
                rs = pps.tile([1, 512], F32, name="rmsps", tag="ps")
                nc.tensor.matmul(rs[:], wvec_sb[r0:r0 + 32, wcol:wcol + 1],
                                 sqs[0][r0:r0 + 32, :], start=True, stop=False,
                                 tile_position=(r0, 0), skip_group_check=True)
                nc.tensor.matmul(rs[:], wvec_sb[r0:r0 + 32, wcol + 1:wcol + 2],
                                 sqs[1][r0:r0 + 32, :], start=False, stop=True,
                                 tile_position=(r0, 0), skip_group_check=True)
                rrow = rcp.tile([1, 512], F32, tag="rrow")
                nc.vector.tensor_copy(rrow[:], rs[:])
                nc.gpsimd.dma_start(
                    out=skb[rowbase + hh:rowbase + hh + 1, ls], in_=rrow[:])
        # V projection: l on partitions
        for ls4 in range(4):
            lt = lc * 4 + ls4
            ps = pps.tile([128, CPG], F32, name="vps", tag="ps")
            for dt in range(NDT):
                nc.tensor.matmul(ps[:], xtb[dt][:, ls4 * 128:(ls4 + 1) * 128],
                                 wv_sb[:, dt, :], start=(dt == 0),
                                 stop=(dt == NDT - 1))
            for h in range(HPG):
                nc.scalar.activation(v_sb[h][:, lt, 0:64],
                                     ps[:, h * 64:(h + 1) * 64], AF.Copy)

        # per-lc r chains
        for h in range(HPG):
            nc.gpsimd.dma_start(out=Rq[32 * h:32 * h + 32, ls],
                              in_=skb[h:h + 1, ls].to_broadcast((32, 512)))
        nc.scalar.activation(Rq[:, ls], Rq[:, ls], AF.Sqrt, bias=epsP[:])
        nc.vector.reciprocal_approx_fast(out=Rq[:, ls], in_=Rq[:, ls])
        kslice = slice(4 * lc, 4 * lc + 4)
        for h in range(HPG):
            nc.gpsimd.dma_start(out=skT[:, kslice, h],
                              in_=skb[4 + h, ls].rearrange("(t p) -> p t", p=128))
        nc.scalar.activation(skT[:, kslice, :], skT[:, kslice, :], AF.Sqrt,
                             bias=eps64P[:])
        nc.vector.reciprocal_approx_fast(out=rkT[:, kslice, :],
                                         in_=skT[:, kslice, :])

        # per-lc RoPE (+ r_q fold on the q side)
        for base in (0, 2):
            xr, xi = qk_sb[base][:, ls], qk_sb[base + 1][:, ls]
            for comp in range(2):
                t1 = tmp.tile([128, 512], F32, tag="t1")
                t2 = tmp.tile([128, 512], F32, tag="t2")
                ca, cb = (cos_sb, sin_sb) if comp == 0 else (sin_sb, cos_sb)
                nc.vector.tensor_tensor(t1[:], xr, ca[:, ls], MUL)
                nc.vector.tensor_tensor(t2[:], xi, cb[:, ls], MUL)
                op = SUB if comp == 0 else ADD
                dst = rope_sb[base + comp][:, ls]
                if base == 0:
                    t3 = tmp.tile([128, 512], F32, tag="t3")
                    nc.vector.tensor_tensor(t3[:], t1[:], t2[:], op)
                    nc.vector.tensor_tensor(dst, t3[:], Rq[:, ls], MUL)
                else:
                    nc.vector.tensor_tensor(dst, t1[:], t2[:], op)

        # per-lc shuffle into per-head contiguous bf16 tiles
        for hp2 in range(2):
            for i2 in range(2):
                h2 = hp2 * 2 + i2
                nc.gpsimd.tensor_copy(QP[hp2][64 * i2:64 * i2 + 32, ls],
                                      rope_sb[0][32 * h2:32 * h2 + 32, ls])
                nc.gpsimd.tensor_copy(QP[hp2][64 * i2 + 32:64 * i2 + 64, ls],
                                      rope_sb[1][32 * h2:32 * h2 + 32, ls])
                nc.gpsimd.tensor_copy(KP[hp2][64 * i2:64 * i2 + 32, ls],
                                      rope_sb[2][32 * h2:32 * h2 + 32, ls])
                nc.gpsimd.tensor_copy(KP[hp2][64 * i2 + 32:64 * i2 + 64, ls],
                                      rope_sb[3][32 * h2:32 * h2 + 32, ls])

    # ---- phase 4: attention (frame pairs, kt-major) ----
    for hp in range(2):
        for fp in range(4):
            f0, f1 = 2 * fp, 2 * fp + 1         # frames in this pair
            nkt_sh, nkt_all = 4 * fp + 2, 4 * fp + 4
            pvps = [[ppv.tile([65, 256], F32, name=f"pv{hp}_{fp}_{i}_{f}",
                              tag="ps") for f in range(2)] for i in range(2)]
            pend = []

            def flush_pv():
                for kt_, i_, pt_ in pend:
                    h_ = hp * 2 + i_
                    if kt_ < nkt_sh:
                        nc.tensor.matmul(pvps[i_][0][:], v_sb[h_][:, kt_, :],
                                         pt_[:, 0:256], start=(kt_ == 0),
                                         stop=(kt_ == nkt_sh - 1),
                                         skip_group_check=True)
                        nc.tensor.matmul(pvps[i_][1][:], v_sb[h_][:, kt_, :],
                                         pt_[:, 256:512], start=(kt_ == 0),
                                         stop=False, skip_group_check=True)
                    else:
                        nc.tensor.matmul(pvps[i_][1][:], v_sb[h_][:, kt_, :],
                                         pt_[:, 0:256], start=False,
                                         stop=(kt_ == nkt_all - 1),
                                         skip_group_check=True)
                pend.clear()

            for kt in range(nkt_all):
                ks = slice(kt * 128, (kt + 1) * 128)
                shared = kt < nkt_sh
                qc = (slice(512 * fp, 512 * fp + 512) if shared
                      else slice(256 * f1, 256 * f1 + 256))
                nq = 512 if shared else 256
                cur = []
                for i in range(2):
                    h = hp * 2 + i
                    st = pps.tile([128, nq], F32, name=f"st{i}_{nq}", tag="ps")
                    nc.tensor.matmul(st[:], KP[hp][64 * i:64 * i + 64, ks],
                                     QP[hp][64 * i:64 * i + 64, qc],
                                     start=True, stop=True,
                                     skip_group_check=True)
                    pt = ptp.tile([128, nq], BF16, name=f"pt{i}_{nq}", tag="pt")
                    nc.scalar.activation(pt[:], st[:], AF.Exp,
                                         scale=rkT[:, kt, h:h + 1])
                    cur.append((kt, i, pt))
                flush_pv()
                pend.extend(cur)
            flush_pv()
            sstage = rbp.tile([1, 1024], F32, tag="sstage")
            for i in range(2):
                h = hp * 2 + i
                for fi, f in enumerate((f0, f1)):
                    qs = slice(f * 256, (f + 1) * 256)
                    dst = att_sb[hp][64 * i:64 * i + 64, qs]
                    srow = sstage[:, (2 * i + fi) * 256:(2 * i + fi + 1) * 256]
                    if (f + i) % 2 == 0:
                        nc.vector.tensor_copy(dst, pvps[i][fi][0:64, :])
                        nc.scalar.activation(srow, pvps[i][fi][64:65, :],
                                             AF.Copy)
                    else:
                        nc.scalar.activation(dst, pvps[i][fi][0:64, :], AF.Copy)
                        nc.vector.tensor_copy(srow, pvps[i][fi][64:65, :])
            nc.sync.dma_start(
                out=skb[8 + 2 * hp:10 + 2 * hp, 512 * fp:512 * fp + 512],
                in_=sstage[:])

        # normalize this head pair while the other pair's attention runs
        Rn = Rq if hp == 0 else Rn1
        for i in range(2):
            h = hp * 2 + i
            nc.gpsimd.dma_start(out=Rn[64 * i:64 * i + 64, :],
                              in_=skb[8 + h:9 + h, :].to_broadcast((64, L)))
        nc.vector.reciprocal_approx_fast(out=Rn[:], in_=Rn[:])
        nc.vector.tensor_tensor(att_sb[hp][:], att_sb[hp][:], Rn[:], MUL)

    # ---- phase 5: output projection ----
    for lt in range(NLT):
        lsl = slice(lt * 128, (lt + 1) * 128)
        for oc in range(2):
            ps = pps.tile([128, 512], F32, name=f"op{lt}_{oc}", tag="ps")
            for ct in range(2):
                nc.tensor.matmul(ps[:], att_sb[ct][:, lsl],
                                 wo_sb[:, ct, oc * 512:(oc + 1) * 512],
                                 start=(ct == 0), stop=(ct == 1))
            ob = osb.tile([128, 512], F32, tag="ob")
            if (lt + oc) % 2 == 0:
                nc.vector.tensor_copy(ob[:], ps[:])
            else:
                nc.scalar.activation(ob[:], ps[:], AF.Copy)
            nc.sync.dma_start(out=out[lsl, oc * 512:(oc + 1) * 512], in_=ob[:])


def _build_nc():
    import contextlib
    nc = bacc.Bacc("TRN2", target_bir_lowering=False, debug=False, num_devices=8)
    xT = nc.dram_tensor("xT", (D, L), F32R, kind="ExternalInput")
    wqk = nc.dram_tensor("wqk", (D, 512), F32R, kind="ExternalInput")
    wv = nc.dram_tensor("wv", (D, CPG), BF16, kind="ExternalInput")
    wo = nc.dram_tensor("wo", (CPG, D), F32R, kind="ExternalInput")
    wvec = nc.dram_tensor("wvec", (128, 4), F32R, kind="ExternalInput")
    costab = nc.dram_tensor("costab", (128, L), F32, kind="ExternalInput")
    sintab = nc.dram_tensor("sintab", (128, L), F32, kind="ExternalInput")
    out = nc.dram_tensor("out", (L, D), F32, kind="ExternalOutput")
    skb = nc.dram_tensor("skb", (12, L), F32)

    with tile.TileContext(nc) as tc, contextlib.ExitStack() as ctx:
        _emit(nc, tc, ctx, xT.ap(), wqk.ap(), wv.ap(), wo.ap(), wvec.ap(),
              costab.ap(), sintab.ap(), out.ap(), skb.ap())
    nc.compile()
    return nc


def _host_prep(x, Wqkv, Wout, q_scale, k_scale):
    x = np.asarray(x, np.float32)
    Wqkv = np.asarray(Wqkv, np.float32)
    Wout = np.asarray(Wout, np.float32)
    q_scale = np.asarray(q_scale, np.float32)
    k_scale = np.asarray(k_scale, np.float32)

    quarter = HD // 4  # 16
    inv = 1.0 / (10000.0 ** (np.arange(quarter, dtype=np.float64) / quarter))
    tt = np.repeat(np.arange(T), NP).astype(np.float64)
    pp = np.tile(np.arange(NP), T).astype(np.float64)
    ang = np.concatenate([tt[:, None] * inv[None, :], pp[:, None] * inv[None, :]],
                         axis=1)  # (L, 32)
    costab = np.tile(np.cos(ang).astype(np.float32).T, (4, 1))  # (128, L)
    sintab = np.tile(np.sin(ang).astype(np.float32).T, (4, 1))

    ev, od = np.arange(0, HD, 2), np.arange(1, HD, 2)
    wvec = np.empty((128, 4), np.float32)
    for hh in range(HPG):
        r = slice(32 * hh, 32 * hh + 32)
        wvec[r, 0] = 1.0 / (HD * q_scale[ev] ** 2)
        wvec[r, 1] = 1.0 / (HD * q_scale[od] ** 2)
        wvec[r, 2] = 1.0 / (k_scale[ev] ** 2)
        wvec[r, 3] = 1.0 / (k_scale[od] ** 2)

    in_maps = []
    for c in range(8):
        b, g = c // 4, c % 4
        wqk = np.empty((D, 512), np.float32)
        for hh in range(HPG):
            gh = g * HPG + hh
            wq = Wqkv[gh * HD:(gh + 1) * HD, :] * q_scale[:, None]
            wk = Wqkv[D + gh * HD:D + (gh + 1) * HD, :] * k_scale[:, None]
            wqk[:, 0 + 32 * hh:32 + 32 * hh] = wq[ev].T
            wqk[:, 128 + 32 * hh:160 + 32 * hh] = wq[od].T
            wqk[:, 256 + 32 * hh:288 + 32 * hh] = wk[ev].T
            wqk[:, 384 + 32 * hh:416 + 32 * hh] = wk[od].T
        import ml_dtypes
        wv = np.ascontiguousarray(
            Wqkv[2 * D + g * CPG:2 * D + (g + 1) * CPG, :].T).astype(ml_dtypes.bfloat16)
        wo = np.ascontiguousarray(Wout[:, g * CPG:(g + 1) * CPG].T)
        in_maps.append({
            "xT": np.ascontiguousarray(x[b].T),
            "wqk": wqk, "wv": wv, "wo": wo, "wvec": wvec,
            "costab": costab, "sintab": sintab,
        })
    return in_maps


def kernel(x, Wqkv, Wout, q_scale, k_scale, T=None, N_p=None):
    assert int(T) == 8 and int(N_p) == 256
    if "nc" not in _CACHE:
        _CACHE["nc"] = _build_nc()
    nc = _CACHE["nc"]
    in_maps = _host_prep(x, Wqkv, Wout, q_scale, k_scale)
    trace = bool(int(os.environ.get("KERNEL_TRACE", "0")))
    res = run_bass_kernel_spmd(nc, in_maps, core_ids=list(range(8)), trace=trace)
    _CACHE["last_exec_time_ns"] = res.exec_time_ns
    outp = np.zeros((B, L, D), np.float32)
    for c in range(8):
        outp[c // 4] += res.results[c]["out"]
    return outp


if __name__ == "__main__":
    rng = np.random.default_rng(0)
    x = rng.standard_normal((B, L, D), dtype=np.float32)
    Wqkv = rng.standard_normal((3 * D, D), dtype=np.float32) * 0.02
    Wout = rng.standard_normal((D, D), dtype=np.float32) * 0.02
    o = kernel(x, Wqkv, Wout, np.ones(HD, np.float32), np.ones(HD, np.float32),
               8, 256)
    print("out", o.shape, o.dtype, float(np.abs(o).mean()))


# revision 21
# speedup vs baseline: 1.2494x; 1.2494x over previous
"""Block-causal attention Trainium2 kernel (8 NeuronCores).

Sharding: core c = b*4 + g handles batch b (of 2) and head-group g (4 of 16
heads). Each core computes the qkv projection, rmsnorm + 2-D RoPE,
block-causal attention and a partial output projection for its 256 channels;
the host sums the 4 per-group partials per batch.

On-chip layouts (per core):
  Q^T/K^T: feature-on-partition tiles QR/QI/KR/KI [128, 2048]; row 32*hh+j
    <-> head hh, complex pair j (R = even orig dim 2j, I = odd 2j+1).
  V: [l, d] tiles per head [128, 16, 65] with an all-ones column 64 so the
    softmax denominator falls out of the M=65 PV matmul.
  Scores: S^T [keys=128, q=256] per (head, frame, ktile); block-causal means
    frame t only attends keys < 256*(t+1) -- no mask tensor anywhere.
  rmsnorm: r = rsqrt(mean(q^2)+eps) via weighted ones-matmul over partitions;
    q_scale/k_scale are folded into the projection weights; the k-side
    0.125*r_k is folded into exp()'s per-partition scale and the q-side r_q
    is multiplied into Q^T during RoPE. exp() needs no max-subtraction
    (|scores| <= 8 after rmsnorm).
All matmuls run in float32r (full PE rate, ~1e-4 component error).
"""

import os
import numpy as np

import concourse.bass as bass
import concourse.mybir as mybir
import concourse.tile as tile
from concourse import bacc
from concourse.bass_utils import run_bass_kernel_spmd

F32 = mybir.dt.float32
F32R = mybir.dt.float32r
BF16 = mybir.dt.bfloat16
AF = mybir.ActivationFunctionType
MUL = mybir.AluOpType.mult
ADD = mybir.AluOpType.add
SUB = mybir.AluOpType.subtract

B, T, NP, D, H = 2, 8, 256, 1024, 16
L = T * NP            # 2048
HD = 64               # head dim
HPG = 4               # heads per group (4 groups x 2 batches = 8 cores)
CPG = HPG * HD        # 256 channels per group
NDT = D // 128        # 8 d-tiles
NLC = L // 512        # 4 l-chunks
NLT = L // 128        # 16 l-tiles
EPS = 1e-6

_CACHE = {}


def _emit(nc, tc, ctx, xT, wqk, wv, wo, wvec, costab, sintab, out, skb):
    sing = ctx.enter_context(tc.tile_pool(name="sing", bufs=1))
    xp = ctx.enter_context(tc.tile_pool(name="xp", bufs=8))
    tmp = ctx.enter_context(tc.tile_pool(name="tmp", bufs=2))
    sqp = ctx.enter_context(tc.tile_pool(name="sqp", bufs=2))
    ptp = ctx.enter_context(tc.tile_pool(name="ptp", bufs=6))
    rbp = ctx.enter_context(tc.tile_pool(name="rbp", bufs=2))
    osb = ctx.enter_context(tc.tile_pool(name="osb", bufs=2))
    # PSUM: one shared transient pool (6 banks, single tag) + PV pool (2)
    pps = ctx.enter_context(tc.tile_pool(name="pps", bufs=8, space="PSUM"))
    ppv = pps

    # ---- persistent SBUF ----
    wqk_sb = sing.tile([128, NDT, 512], BF16)
    nc.sync.dma_start(out=wqk_sb[:], in_=wqk.rearrange("(t p) o -> p t o", p=128))
    wv_sb = sing.tile([128, NDT, CPG], BF16)
    nc.sync.dma_start(out=wv_sb[:], in_=wv.rearrange("(t p) o -> p t o", p=128))
    wo_sb = sing.tile([128, 2, D], BF16)
    nc.sync.dma_start(out=wo_sb[:], in_=wo.rearrange("(t p) o -> p t o", p=128))
    wvec_sb = sing.tile([128, 4], F32R)
    nc.sync.dma_start(out=wvec_sb[:], in_=wvec[:])
    cos_sb = sing.tile([128, L], F32)
    nc.sync.dma_start(out=cos_sb[:], in_=costab[:])
    sin_sb = sing.tile([128, L], F32)
    nc.sync.dma_start(out=sin_sb[:], in_=sintab[:])

    qk_sb = [sing.tile([128, L], BF16, name=f"qk{i}") for i in range(4)]
    rope_sb = [sing.tile([128, L], BF16, name=f"rope{i}") for i in range(4)]
    v_sb = [sing.tile([128, NLT, 65], BF16, name=f"v{h}") for h in range(HPG)]
    att_sb = [sing.tile([128, L], BF16, name=f"att{i}") for i in range(2)]
    ones_f32 = sing.tile([128, NLT, 1], F32)
    nc.vector.memset(ones_f32[:], 1.0)
    for h in range(HPG):
        nc.vector.tensor_copy(v_sb[h][:, :, 64:65], ones_f32[:])

    epsP = sing.tile([128, 1], F32)
    nc.vector.memset(epsP[:], EPS)
    eps64P = sing.tile([128, 1], F32)
    nc.vector.memset(eps64P[:], 64.0 * EPS)
    rcp = ctx.enter_context(tc.tile_pool(name="rcp", bufs=2))
    Rq = sing.tile([128, L], F32)
    Rn1 = sing.tile([128, L], F32)
    skT = sing.tile([128, NLT, 4], F32)
    rkT = sing.tile([128, NLT, 4], F32)

    QP = [sing.tile([128, L], BF16, name=f"qp{i}") for i in range(2)]
    KP = [sing.tile([128, L], BF16, name=f"kp{i}") for i in range(2)]
    QRr, QIr, KRr, KIr = rope_sb

    # ---- phase 1: projections + rms partition-sums ----
    for lc in range(NLC):
        ls = slice(lc * 512, (lc + 1) * 512)
        xt = []
        for dt in range(NDT):
            x1 = xp.tile([128, 512], BF16, name=f"xt{dt}", tag="xt")
            nc.sync.dma_start(out=x1[:], in_=xT[dt * 128:(dt + 1) * 128, ls])
            xt.append(x1)
        for pair, rowbase, wcol in ((0, 0, 0), (2, 4, 2)):
            sqs = []
            for comp in range(2):           # R then I
                ot = pair + comp
                ps = pps.tile([128, 512], F32, name="qkps", tag="ps")
                for dt in range(NDT):
                    nc.tensor.matmul(ps[:], wqk_sb[:, dt, ot * 128:(ot + 1) * 128],
                                     xt[dt][:], start=(dt == 0), stop=(dt == NDT - 1))
                nc.vector.tensor_copy(qk_sb[ot][:, ls], ps[:])
                sq = sqp.tile([128, 512], F32R, tag="sq")
                nc.scalar.activation(sq[:], qk_sb[ot][:, ls], AF.Square)
                sqs.append(sq)
            for hh in range(HPG):
                r0 = 32 * hh
                rs = pps.tile([1, 512], F32, name="rmsps", tag="ps")
                nc.tensor.matmul(rs[:], wvec_sb[r0:r0 + 32, wcol:wcol + 1],
                                 sqs[0][r0:r0 + 32, :], start=True, stop=False,
                                 tile_position=(r0, 0), skip_group_check=True)
                nc.tensor.matmul(rs[:], wvec_sb[r0:r0 + 32, wcol + 1:wcol + 2],
                                 sqs[1][r0:r0 + 32, :], start=False, stop=True,
                                 tile_position=(r0, 0), skip_group_check=True)
                rrow = rcp.tile([1, 512], F32, tag="rrow")
                nc.vector.tensor_copy(rrow[:], rs[:])
                nc.gpsimd.dma_start(
                    out=skb[rowbase + hh:rowbase + hh + 1, ls], in_=rrow[:])
        # V projection: l on partitions
        for ls4 in range(4):
            lt = lc * 4 + ls4
            ps = pps.tile([128, CPG], F32, name="vps", tag="ps")
            for dt in range(NDT):
                nc.tensor.matmul(ps[:], xt[dt][:, ls4 * 128:(ls4 + 1) * 128],
                                 wv_sb[:, dt, :], start=(dt == 0),
                                 stop=(dt == NDT - 1))
            for h in range(HPG):
                nc.scalar.activation(v_sb[h][:, lt, 0:64],
                                     ps[:, h * 64:(h + 1) * 64], AF.Copy)

        # per-lc r chains
        for h in range(HPG):
            nc.gpsimd.dma_start(out=Rq[32 * h:32 * h + 32, ls],
                              in_=skb[h:h + 1, ls].to_broadcast((32, 512)))
        nc.scalar.activation(Rq[:, ls], Rq[:, ls], AF.Sqrt, bias=epsP[:])
        nc.vector.reciprocal_approx_fast(out=Rq[:, ls], in_=Rq[:, ls])
        kslice = slice(4 * lc, 4 * lc + 4)
        for h in range(HPG):
            nc.gpsimd.dma_start(out=skT[:, kslice, h],
                              in_=skb[4 + h, ls].rearrange("(t p) -> p t", p=128))
        nc.scalar.activation(skT[:, kslice, :], skT[:, kslice, :], AF.Sqrt,
                             bias=eps64P[:])
        nc.vector.reciprocal_approx_fast(out=rkT[:, kslice, :],
                                         in_=skT[:, kslice, :])

        # per-lc RoPE (+ r_q fold on the q side)
        for base in (0, 2):
            xr, xi = qk_sb[base][:, ls], qk_sb[base + 1][:, ls]
            for comp in range(2):
                t1 = tmp.tile([128, 512], F32, tag="t1")
                t2 = tmp.tile([128, 512], F32, tag="t2")
                ca, cb = (cos_sb, sin_sb) if comp == 0 else (sin_sb, cos_sb)
                nc.vector.tensor_tensor(t1[:], xr, ca[:, ls], MUL)
                nc.vector.tensor_tensor(t2[:], xi, cb[:, ls], MUL)
                op = SUB if comp == 0 else ADD
                dst = rope_sb[base + comp][:, ls]
                if base == 0:
                    t3 = tmp.tile([128, 512], F32, tag="t3")
                    nc.vector.tensor_tensor(t3[:], t1[:], t2[:], op)
                    nc.vector.tensor_tensor(dst, t3[:], Rq[:, ls], MUL)
                else:
                    nc.vector.tensor_tensor(dst, t1[:], t2[:], op)

        # per-lc shuffle into per-head contiguous bf16 tiles
        for hp2 in range(2):
            for i2 in range(2):
                h2 = hp2 * 2 + i2
                nc.scalar.dma_start(out=QP[hp2][64 * i2:64 * i2 + 32, ls],
                                    in_=rope_sb[0][32 * h2:32 * h2 + 32, ls])
                nc.scalar.dma_start(out=QP[hp2][64 * i2 + 32:64 * i2 + 64, ls],
                                    in_=rope_sb[1][32 * h2:32 * h2 + 32, ls])
                nc.scalar.dma_start(out=KP[hp2][64 * i2:64 * i2 + 32, ls],
                                    in_=rope_sb[2][32 * h2:32 * h2 + 32, ls])
                nc.scalar.dma_start(out=KP[hp2][64 * i2 + 32:64 * i2 + 64, ls],
                                    in_=rope_sb[3][32 * h2:32 * h2 + 32, ls])

    # ---- phase 4: attention (frame pairs, kt-major) ----
    for hp in range(2):
        for fp in range(4):
            f0, f1 = 2 * fp, 2 * fp + 1         # frames in this pair
            nkt_sh, nkt_all = 4 * fp + 2, 4 * fp + 4
            pvps = [[ppv.tile([65, 256], F32, name=f"pv{hp}_{fp}_{i}_{f}",
                              tag="ps") for f in range(2)] for i in range(2)]
            pend = []

            def flush_pv():
                for kt_, i_, pt_ in pend:
                    h_ = hp * 2 + i_
                    if kt_ < nkt_sh:
                        nc.tensor.matmul(pvps[i_][0][:], v_sb[h_][:, kt_, :],
                                         pt_[:, 0:256], start=(kt_ == 0),
                                         stop=(kt_ == nkt_sh - 1),
                                         skip_group_check=True)
                        nc.tensor.matmul(pvps[i_][1][:], v_sb[h_][:, kt_, :],
                                         pt_[:, 256:512], start=(kt_ == 0),
                                         stop=False, skip_group_check=True)
                    else:
                        nc.tensor.matmul(pvps[i_][1][:], v_sb[h_][:, kt_, :],
                                         pt_[:, 0:256], start=False,
                                         stop=(kt_ == nkt_all - 1),
                                         skip_group_check=True)
                pend.clear()

            for kt in range(nkt_all):
                ks = slice(kt * 128, (kt + 1) * 128)
                shared = kt < nkt_sh
                qc = (slice(512 * fp, 512 * fp + 512) if shared
                      else slice(256 * f1, 256 * f1 + 256))
                nq = 512 if shared else 256
                cur = []
                for i in range(2):
                    h = hp * 2 + i
                    st = pps.tile([128, nq], F32, name=f"st{i}_{nq}", tag="ps")
                    nc.tensor.matmul(st[:], KP[hp][64 * i:64 * i + 64, ks],
                                     QP[hp][64 * i:64 * i + 64, qc],
                                     start=True, stop=True,
                                     skip_group_check=True)
                    pt = ptp.tile([128, nq], BF16, name=f"pt{i}_{nq}", tag="pt")
                    nc.scalar.activation(pt[:], st[:], AF.Exp,
                                         scale=rkT[:, kt, h:h + 1])
                    cur.append((kt, i, pt))
                flush_pv()
                pend.extend(cur)
            flush_pv()
            sstage = rbp.tile([1, 1024], F32, tag="sstage")
            for i in range(2):
                h = hp * 2 + i
                for fi, f in enumerate((f0, f1)):
                    qs = slice(f * 256, (f + 1) * 256)
                    dst = att_sb[hp][64 * i:64 * i + 64, qs]
                    srow = sstage[:, (2 * i + fi) * 256:(2 * i + fi + 1) * 256]
                    if (f + i) % 2 == 0:
                        nc.vector.tensor_copy(dst, pvps[i][fi][0:64, :])
                        nc.scalar.activation(srow, pvps[i][fi][64:65, :],
                                             AF.Copy)
                    else:
                        nc.scalar.activation(dst, pvps[i][fi][0:64, :], AF.Copy)
                        nc.vector.tensor_copy(srow, pvps[i][fi][64:65, :])
            nc.sync.dma_start(
                out=skb[8 + 2 * hp:10 + 2 * hp, 512 * fp:512 * fp + 512],
                in_=sstage[:])

        # normalize this head pair while the other pair's attention runs
        Rn = Rq if hp == 0 else Rn1
        for i in range(2):
            h = hp * 2 + i
            nc.gpsimd.dma_start(out=Rn[64 * i:64 * i + 64, :],
                              in_=skb[8 + h:9 + h, :].to_broadcast((64, L)))
        nc.vector.reciprocal_approx_fast(out=Rn[:], in_=Rn[:])
        nc.vector.tensor_tensor(att_sb[hp][:], att_sb[hp][:], Rn[:], MUL)

    # ---- phase 5: output projection ----
    for lt in range(NLT):
        lsl = slice(lt * 128, (lt + 1) * 128)
        for oc in range(2):
            ps = pps.tile([128, 512], F32, name=f"op{lt}_{oc}", tag="ps")
            for ct in range(2):
                nc.tensor.matmul(ps[:], att_sb[ct][:, lsl],
                                 wo_sb[:, ct, oc * 512:(oc + 1) * 512],
                                 start=(ct == 0), stop=(ct == 1))
            ob = osb.tile([128, 512], F32, tag="ob")
            if (lt + oc) % 2 == 0:
                nc.vector.tensor_copy(ob[:], ps[:])
            else:
                nc.scalar.activation(ob[:], ps[:], AF.Copy)
            nc.sync.dma_start(out=out[lsl, oc * 512:(oc + 1) * 512], in_=ob[:])


def _build_nc():
    import contextlib
    nc = bacc.Bacc("TRN2", target_bir_lowering=False, debug=False, num_devices=8)
    xT = nc.dram_tensor("xT", (D, L), BF16, kind="ExternalInput")
    wqk = nc.dram_tensor("wqk", (D, 512), BF16, kind="ExternalInput")
    wv = nc.dram_tensor("wv", (D, CPG), BF16, kind="ExternalInput")
    wo = nc.dram_tensor("wo", (CPG, D), BF16, kind="ExternalInput")
    wvec = nc.dram_tensor("wvec", (128, 4), F32R, kind="ExternalInput")
    costab = nc.dram_tensor("costab", (128, L), F32, kind="ExternalInput")
    sintab = nc.dram_tensor("sintab", (128, L), F32, kind="ExternalInput")
    out = nc.dram_tensor("out", (L, D), F32, kind="ExternalOutput")
    skb = nc.dram_tensor("skb", (12, L), F32)

    with tile.TileContext(nc) as tc, contextlib.ExitStack() as ctx:
        _emit(nc, tc, ctx, xT.ap(), wqk.ap(), wv.ap(), wo.ap(), wvec.ap(),
              costab.ap(), sintab.ap(), out.ap(), skb.ap())
    nc.compile()
    return nc


def _host_prep(x, Wqkv, Wout, q_scale, k_scale):
    x = np.asarray(x, np.float32)
    Wqkv = np.asarray(Wqkv, np.float32)
    Wout = np.asarray(Wout, np.float32)
    q_scale = np.asarray(q_scale, np.float32)
    k_scale = np.asarray(k_scale, np.float32)

    quarter = HD // 4  # 16
    inv = 1.0 / (10000.0 ** (np.arange(quarter, dtype=np.float64) / quarter))
    tt = np.repeat(np.arange(T), NP).astype(np.float64)
    pp = np.tile(np.arange(NP), T).astype(np.float64)
    ang = np.concatenate([tt[:, None] * inv[None, :], pp[:, None] * inv[None, :]],
                         axis=1)  # (L, 32)
    costab = np.tile(np.cos(ang).astype(np.float32).T, (4, 1))  # (128, L)
    sintab = np.tile(np.sin(ang).astype(np.float32).T, (4, 1))

    import ml_dtypes
    ev, od = np.arange(0, HD, 2), np.arange(1, HD, 2)
    wvec = np.empty((128, 4), np.float32)
    for hh in range(HPG):
        r = slice(32 * hh, 32 * hh + 32)
        wvec[r, 0] = 1.0 / (HD * q_scale[ev] ** 2)
        wvec[r, 1] = 1.0 / (HD * q_scale[od] ** 2)
        wvec[r, 2] = 1.0 / (k_scale[ev] ** 2)
        wvec[r, 3] = 1.0 / (k_scale[od] ** 2)

    in_maps = []
    for c in range(8):
        b, g = c // 4, c % 4
        wqk = np.empty((D, 512), np.float32)
        for hh in range(HPG):
            gh = g * HPG + hh
            wq = Wqkv[gh * HD:(gh + 1) * HD, :] * q_scale[:, None]
            wk = Wqkv[D + gh * HD:D + (gh + 1) * HD, :] * k_scale[:, None]
            wqk[:, 0 + 32 * hh:32 + 32 * hh] = wq[ev].T
            wqk[:, 128 + 32 * hh:160 + 32 * hh] = wq[od].T
            wqk[:, 256 + 32 * hh:288 + 32 * hh] = wk[ev].T
            wqk[:, 384 + 32 * hh:416 + 32 * hh] = wk[od].T
        wv = np.ascontiguousarray(
            Wqkv[2 * D + g * CPG:2 * D + (g + 1) * CPG, :].T).astype(ml_dtypes.bfloat16)
        wo = np.ascontiguousarray(Wout[:, g * CPG:(g + 1) * CPG].T)
        in_maps.append({
            "xT": np.ascontiguousarray(x[b].T).astype(ml_dtypes.bfloat16),
            "wqk": wqk.astype(ml_dtypes.bfloat16), "wv": wv,
            "wo": wo.astype(ml_dtypes.bfloat16), "wvec": wvec,
            "costab": costab, "sintab": sintab,
        })
    return in_maps


def kernel(x, Wqkv, Wout, q_scale, k_scale, T=None, N_p=None):
    assert int(T) == 8 and int(N_p) == 256
    if "nc" not in _CACHE:
        _CACHE["nc"] = _build_nc()
    nc = _CACHE["nc"]
    in_maps = _host_prep(x, Wqkv, Wout, q_scale, k_scale)
    trace = bool(int(os.environ.get("KERNEL_TRACE", "0")))
    res = run_bass_kernel_spmd(nc, in_maps, core_ids=list(range(8)), trace=trace)
    _CACHE["last_exec_time_ns"] = res.exec_time_ns
    outp = np.zeros((B, L, D), np.float32)
    for c in range(8):
        outp[c // 4] += res.results[c]["out"]
    return outp


if __name__ == "__main__":
    rng = np.random.default_rng(0)
    x = rng.standard_normal((B, L, D), dtype=np.float32)
    Wqkv = rng.standard_normal((3 * D, D), dtype=np.float32) * 0.02
    Wout = rng.standard_normal((D, D), dtype=np.float32) * 0.02
    o = kernel(x, Wqkv, Wout, np.ones(HD, np.float32), np.ones(HD, np.float32),
               8, 256)
    print("out", o.shape, o.dtype, float(np.abs(o).mean()))
